# revision 18
# baseline (speedup 1.0000x reference)
"""HadamardAttention Trainium2 kernel — 8-core data-parallel over batch.

Per core (one batch element b), everything in "transposed" activation
layout [C on partitions, N on free dim]:

  phase A: qT/kT projections -> Hadamard product -> per-head reduction
           (selector matmul, SCALE folded in) -> tiny MLP -> masked
           scores awFull [H, N] (fp32)
  phase B: softmax over N (free dim) on [32, 4096]
  phase C: vT projection (x re-streamed), head-broadcast of weights
           (selector matmul), p2T = aw*vT, final out-projection which
           naturally restores natural [N, C] layout (p2T is the lhsT).

Host-side prep is layout-only (transpose/reshape) plus dtype casts to
bf16 for TensorE operands; all FLOPs happen on device.
"""
import sys

if "/opt/trn_rl_repo" not in sys.path:
    sys.path.insert(0, "/opt/trn_rl_repo")

import numpy as np
import ml_dtypes
from contextlib import ExitStack

import concourse.bass as bass
import concourse.bacc as bacc
import concourse.tile as tile
from concourse import mybir
from concourse.bass_utils import run_bass_kernel_spmd

# antenv.axon_hooks is absent in some images; shim it so trace=True can
# reach the NTFF profiler. Harmless no-op for trace=False runs.
try:
    from antenv.axon_hooks import get_axon_ntff_profile_hook  # noqa: F401
except ImportError:
    try:
        import types
        import antenv

        _hooks = types.ModuleType("antenv.axon_hooks")
        _hooks._hook = None
        _hooks.set_axon_ntff_profile_hook = lambda h: setattr(_hooks, "_hook", h)
        _hooks.get_axon_ntff_profile_hook = lambda: _hooks._hook
        sys.modules["antenv.axon_hooks"] = _hooks
        antenv.axon_hooks = _hooks
        from trn_agent_boot.trn_boot import _ntff_profile_via_ctypes

        _hooks.set_axon_ntff_profile_hook(
            _ntff_profile_via_ctypes("/opt/axon/libaxon_pjrt.so"))
    except Exception:
        pass

B, N, C, H, D = 8, 4096, 1024, 32, 32
SCALE = float(D) ** -0.5
P = 128
CK = C // P          # 8 chunks of the channel dim
NQ = 4               # token quarters
TQ = N // NQ         # 1024 tokens per quarter
TC = 512             # moving free dim per matmul
BF16 = mybir.dt.bfloat16
F32 = mybir.dt.float32
I32 = mybir.dt.int32
AF = mybir.ActivationFunctionType
ALU = mybir.AluOpType


def _build():
    nc = bacc.Bacc("TRN2", num_devices=8)

    xTr = nc.declare_dram_parameter("xTr", [P, CK, N], BF16, isOutput=False)
    mask = nc.declare_dram_parameter("mask", [1, N], I32, isOutput=False)
    wq = nc.declare_dram_parameter("wq", [P, CK, C], BF16, isOutput=False)
    wk = nc.declare_dram_parameter("wk", [P, CK, C], BF16, isOutput=False)
    wv = nc.declare_dram_parameter("wv", [P, CK, C], BF16, isOutput=False)
    wo = nc.declare_dram_parameter("wo", [P, CK, C], BF16, isOutput=False)
    w1 = nc.declare_dram_parameter("w1", [H, 2 * D], BF16, isOutput=False)
    w2 = nc.declare_dram_parameter("w2", [2 * D, H], BF16, isOutput=False)
    bq = nc.declare_dram_parameter("bq", [P, CK], F32, isOutput=False)
    bk = nc.declare_dram_parameter("bk", [P, CK], F32, isOutput=False)
    bv = nc.declare_dram_parameter("bv", [P, CK], F32, isOutput=False)
    b1 = nc.declare_dram_parameter("b1", [2 * D, 1], F32, isOutput=False)
    b2 = nc.declare_dram_parameter("b2", [H, 1], F32, isOutput=False)
    bo = nc.declare_dram_parameter("bo", [1, C], BF16, isOutput=False)
    sel1 = nc.declare_dram_parameter("sel1", [P, CK, H], BF16, isOutput=False)
    sel2 = nc.declare_dram_parameter("sel2", [H, CK, P], BF16, isOutput=False)
    ones = nc.declare_dram_parameter("ones", [1, P], BF16, isOutput=False)
    out = nc.declare_dram_parameter("out", [N, C], F32, isOutput=True)

    with tile.TileContext(nc) as tc:
        with ExitStack() as ctx:
            wpool = ctx.enter_context(tc.tile_pool(name="wpool", bufs=1))
            const = ctx.enter_context(tc.tile_pool(name="const", bufs=1))
            narrow = ctx.enter_context(tc.tile_pool(name="narrow", bufs=1))
            small = ctx.enter_context(tc.tile_pool(name="small", bufs=2))
            xin = ctx.enter_context(tc.tile_pool(name="xin", bufs=2))
            ppool = ctx.enter_context(tc.tile_pool(name="ppool", bufs=2))
            qv = ctx.enter_context(tc.tile_pool(name="qv", bufs=4))
            ypool = ctx.enter_context(tc.tile_pool(name="ypool", bufs=4))
            bank = ctx.enter_context(
                tc.tile_pool(name="bank", bufs=5, space="PSUM"))
            sbank = ctx.enter_context(
                tc.tile_pool(name="sbank", bufs=3, space="PSUM"))

            # ---- constants / weights -------------------------------------
            wq_sb = wpool.tile([P, CK, C], BF16, tag="wq")
            wk_sb = wpool.tile([P, CK, C], BF16, tag="wk")
            wv_sb = wpool.tile([P, CK, C], BF16, tag="wv")
            wo_sb = wpool.tile([P, CK, C], BF16, tag="wo")
            # startup-critical loads first, interleaved per-kc so the first
            # accumulation group's operands land ASAP: xt(q0) + wq, then wk.
            xt0 = xin.tile([P, CK, TQ], BF16, tag="xin", name="xt0")
            for kc in range(CK):
                nc.sync.dma_start(out=xt0[:, kc, :TC], in_=xTr[:, kc, 0:TC])
                nc.sync.dma_start(out=wq_sb[:, kc, :], in_=wq[:, kc, :])
            for kc in range(CK):
                nc.sync.dma_start(out=wk_sb[:, kc, :], in_=wk[:, kc, :])
            w1_sb = const.tile([H, 2 * D], BF16, tag="w1")
            w2_sb = const.tile([2 * D, H], BF16, tag="w2")
            sel1_sb = const.tile([P, CK, H], BF16, tag="sel1")
            sel2_sb = const.tile([H, CK, P], BF16, tag="sel2")
            bq_sb = const.tile([P, CK], F32, tag="bq")
            bk_sb = const.tile([P, CK], F32, tag="bk")
            bv_sb = const.tile([P, CK], F32, tag="bv")
            b1_sb = const.tile([2 * D, 1], F32, tag="b1")
            b2_sb = const.tile([H, 1], F32, tag="b2")
            bo_sb = const.tile([1, C], BF16, tag="bo")
            ones_sb = const.tile([1, P], BF16, tag="ones")
            for t_sb, t_dr in ((w1_sb, w1), (w2_sb, w2), (sel1_sb, sel1),
                               (sel2_sb, sel2), (bq_sb, bq), (bk_sb, bk),
                               (bv_sb, bv), (b1_sb, b1), (b2_sb, b2),
                               (bo_sb, bo), (ones_sb, ones)):
                nc.sync.dma_start(out=t_sb[:], in_=t_dr[:])

            # additive mask row: 0 where mask==1, -1e9 where mask==0.
            # mask_sb borrows an xin slot (same byte size as the bf16 tiles).
            mask_sb = xin.tile([P, CK, TQ // 2], I32, tag="xin")
            mask_flat = mask_sb[:1, :, :].rearrange("p a b -> p (a b)")
            nc.sync.dma_start(out=mask_flat[:, :N], in_=mask[:, :])
            madd = narrow.tile([1, N], BF16, tag="madd")
            nc.vector.tensor_scalar(
                out=madd[:], in0=mask_flat[:, :N],
                scalar1=1e9, scalar2=-1e9, op0=ALU.mult, op1=ALU.add)

            # bo replicated across partitions via a step-0 DMA broadcast,
            # so the out-projection bias is a DVE add (not PE matmuls).
            bo_rep = const.tile([P, C], BF16, tag="bo_rep")
            bo_ap = bo[:, :]
            bo_bcast = bass.AP(tensor=bo_ap.tensor, offset=bo_ap.offset,
                               ap=[[0, P], list(bo_ap.ap)[1]])
            nc.gpsimd.dma_start(out=bo_rep[:], in_=bo_bcast)

            awFull = narrow.tile([H, N], F32, tag="awFull")
            awn = narrow.tile([H, N], BF16, tag="awn")
            maxP = narrow.tile([H, 2 * NQ], F32, tag="maxP")
            sumP = narrow.tile([H, 2 * NQ], F32, tag="sumP")
            negmax = narrow.tile([H, 1], F32, tag="negmax")
            sums = narrow.tile([H, 1], F32, tag="sums")
            inv = narrow.tile([H, 1], F32, tag="inv")

            # ---- phase A: scores ----------------------------------------
            # first two segments are 512 tokens so the opening accumulation
            # group only waits for wq + 1MB of x (startup latency).
            SEGS = [(0, TC), (TC, TC), (TQ, TQ), (2 * TQ, TQ), (3 * TQ, TQ)]
            for s0, ln in SEGS:
                nt = ln // TC
                if s0 == 0:
                    xt = xt0
                else:
                    xt = xin.tile([P, CK, TQ], BF16, tag="xin",
                                  name=f"xtA_{s0}")
                    for kc in range(CK):
                        nc.sync.dma_start(out=xt[:, kc, :ln],
                                          in_=xTr[:, kc, s0:s0 + ln])
                pT = ppool.tile([P, CK, TQ], BF16, tag="pT",
                                name=f"pT_{s0}")
                for mc in range(CK):
                    ps_q = [bank.tile([P, TC], F32, tag="bank",
                                      name=f"psq_{s0}_{mc}_{t}")
                            for t in range(nt)]
                    for kc in range(CK):
                        for t in range(nt):
                            nc.tensor.matmul(
                                ps_q[t][:],
                                wq_sb[:, kc, mc * P:(mc + 1) * P],
                                xt[:, kc, t * TC:(t + 1) * TC],
                                start=(kc == 0), stop=(kc == CK - 1))
                    q_mc = qv.tile([P, 2, TC], BF16, tag="qv",
                                   name=f"qmc_{s0}_{mc}")
                    for t in range(nt):
                        nc.scalar.activation(
                            q_mc[:, t, :], ps_q[t][:], AF.Identity,
                            bias=bq_sb[:, mc:mc + 1])
                    ps_k = [bank.tile([P, TC], F32, tag="bank",
                                      name=f"psk_{s0}_{mc}_{t}")
                            for t in range(nt)]
                    for kc in range(CK):
                        for t in range(nt):
                            nc.tensor.matmul(
                                ps_k[t][:],
                                wk_sb[:, kc, mc * P:(mc + 1) * P],
                                xt[:, kc, t * TC:(t + 1) * TC],
                                start=(kc == 0), stop=(kc == CK - 1))
                    for t in range(nt):
                        nc.vector.scalar_tensor_tensor(
                            out=pT[:, mc, t * TC:(t + 1) * TC],
                            in0=ps_k[t][:], scalar=bk_sb[:, mc:mc + 1],
                            in1=q_mc[:, t, :], op0=ALU.add, op1=ALU.mult)
                for t in range(nt):
                    j = (s0 + t * TC) // TC
                    ps_aw = sbank.tile([2 * D, TC], F32, tag="sbank")
                    for ci in range(CK):
                        nc.tensor.matmul(
                            ps_aw[:H, :], sel1_sb[:, ci, :],
                            pT[:, ci, t * TC:(t + 1) * TC],
                            start=(ci == 0), stop=(ci == CK - 1))
                    aw0 = small.tile([H, TC], BF16, tag="aw0")
                    nc.scalar.activation(aw0[:], ps_aw[:H, :], AF.Copy)
                    ps_a1 = sbank.tile([2 * D, TC], F32, tag="sbank")
                    nc.tensor.matmul(ps_a1[:], w1_sb[:], aw0[:],
                                     start=True, stop=True)
                    a1 = small.tile([2 * D, TC], BF16, tag="a1")
                    nc.scalar.activation(a1[:], ps_a1[:], AF.Relu,
                                         bias=b1_sb[:])
                    ps_aw2 = sbank.tile([2 * D, TC], F32, tag="sbank")
                    nc.tensor.matmul(ps_aw2[:H, :], w2_sb[:], a1[:],
                                     start=True, stop=False)
                    nc.tensor.matmul(ps_aw2[:H, :], ones_sb[:1, :H],
                                     madd[:1, j * TC:(j + 1) * TC],
                                     start=False, stop=True)
                    nc.scalar.activation(
                        awFull[:, j * TC:(j + 1) * TC], ps_aw2[:H, :],
                        AF.Identity, bias=b2_sb[:])
                    # flash-style: per-chunk -max, then exp in place
                    # with that max; global correction happens in phase B.
                    nc.vector.reduce_max(
                        out=maxP[:, j:j + 1],
                        in_=awFull[:, j * TC:(j + 1) * TC],
                        axis=mybir.AxisListType.X, negate=True)
                    nc.scalar.activation(awFull[:, j * TC:(j + 1) * TC],
                                         awFull[:, j * TC:(j + 1) * TC],
                                         AF.Exp, bias=maxP[:, j:j + 1],
                                         accum_out=sumP[:, j:j + 1])

            # ---- phase B: flash-softmax correction (tiny) ---------------
            # maxP holds -chunkmax; global negmax = min_j maxP[j].
            nc.vector.tensor_reduce(out=negmax[:], in_=maxP[:],
                                    axis=mybir.AxisListType.X,
                                    op=ALU.min)
            # corr[j] = exp(chunkmax_j - globalmax) = exp(-maxP_j + negmax)
            corr = narrow.tile([H, 2 * NQ], F32, tag="corr")
            nc.scalar.activation(corr[:], maxP[:], AF.Exp,
                                 bias=negmax[:], scale=-1.0)
            wsum = narrow.tile([H, 2 * NQ], F32, tag="wsum")
            nc.vector.tensor_mul(wsum[:], sumP[:], corr[:])
            nc.vector.reduce_sum(out=sums[:], in_=wsum[:],
                                 axis=mybir.AxisListType.X)
            nc.vector.reciprocal(out=inv[:], in_=sums[:])
            sfac = narrow.tile([H, 2 * NQ], F32, tag="sfac")
            nc.vector.tensor_scalar(out=sfac[:], in0=corr[:],
                                    scalar1=inv[:], scalar2=None,
                                    op0=ALU.mult)
            for j in range(2 * NQ):
                nc.scalar.activation(awn[:, j * TC:(j + 1) * TC],
                                     awFull[:, j * TC:(j + 1) * TC],
                                     AF.Copy, scale=sfac[:, j:j + 1])

            # ---- phase C: v, weighting, out-projection ------------------
            # wv/wo stream in during phase A compute
            for t_sb, t_dr in ((wv_sb, wv), (wo_sb, wo)):
                for kc in range(CK):
                    nc.sync.dma_start(out=t_sb[:, kc, :],
                                      in_=t_dr[:, kc, :])
            for iq in range(NQ):
                xt = xin.tile([P, CK, TQ], BF16, tag="xin")
                for kc in range(CK):
                    nc.sync.dma_start(
                        out=xt[:, kc, :],
                        in_=xTr[:, kc, iq * TQ:(iq + 1) * TQ])
                p2 = ppool.tile([P, CK, TQ], BF16, tag="pT")
                # software pipeline: v-matmuls for mc run 2 iterations ahead
                # of the awn-dependent awb/p2 stage, so the PE has ~32 MMs
                # of cover while the softmax chain finishes.
                psv = {}

                def v_stage(mc, iq=iq, xt=xt, psv=psv):
                    psv[mc] = [bank.tile([P, TC], F32, tag="bank",
                                         name=f"psv_{iq}_{mc}_{t}")
                               for t in range(2)]
                    for kc in range(CK):
                        for t in range(2):
                            nc.tensor.matmul(
                                psv[mc][t][:],
                                wv_sb[:, kc, mc * P:(mc + 1) * P],
                                xt[:, kc, t * TC:(t + 1) * TC],
                                start=(kc == 0), stop=(kc == CK - 1))

                def awb_stage(mc, iq=iq, p2=p2, psv=psv):
                    awb_mc = qv.tile([P, 2, TC], BF16, tag="qv",
                                     name=f"awb_{iq}_{mc}")
                    for t in range(2):
                        j = iq * 2 + t
                        ps_awb = sbank.tile([P, TC], F32, tag="sbank",
                                            name=f"psawb_{iq}_{mc}_{t}")
                        nc.tensor.matmul(
                            ps_awb[:], sel2_sb[:, mc, :],
                            awn[:, j * TC:(j + 1) * TC],
                            start=True, stop=True)
                        nc.vector.tensor_copy(out=awb_mc[:, t, :],
                                              in_=ps_awb[:])
                    for t in range(2):
                        nc.vector.scalar_tensor_tensor(
                            out=p2[:, mc, t * TC:(t + 1) * TC],
                            in0=psv[mc][t][:], scalar=bv_sb[:, mc:mc + 1],
                            in1=awb_mc[:, t, :], op0=ALU.add, op1=ALU.mult)
                    del psv[mc]

                LOOKAHEAD = 1
                for mc in range(CK):
                    v_stage(mc)
                    if mc >= LOOKAHEAD:
                        awb_stage(mc - LOOKAHEAD)
                for mc in range(CK - LOOKAHEAD, CK):
                    awb_stage(mc)
                for nt in range(TQ // P):
                    n0 = iq * TQ + nt * P
                    for co in range(2):
                        ps_y = bank.tile([P, TC], F32, tag="bank")
                        for ci in range(CK):
                            nc.tensor.matmul(
                                ps_y[:], p2[:, ci, nt * P:(nt + 1) * P],
                                wo_sb[:, ci, co * TC:(co + 1) * TC],
                                start=(ci == 0), stop=(ci == CK - 1))
                        y_sb = ypool.tile([P, TC], F32, tag="y")
                        nc.vector.tensor_add(
                            y_sb[:], ps_y[:],
                            bo_rep[:, co * TC:(co + 1) * TC])
                        nc.sync.dma_start(
                            out=out[n0:n0 + P, co * TC:(co + 1) * TC],
                            in_=y_sb[:])
    nc.finalize()
    return nc


def _prep_core_inputs(b, x, mask, Wq, bq, Wk, bk, Wv, bv, W1, b1, W2, b2,
                      Wo, bo, sel1, sel2, ones_r):
    bf = ml_dtypes.bfloat16
    xT = np.ascontiguousarray(x[b].T).astype(bf)            # [C, N]
    xTr = np.ascontiguousarray(xT.reshape(CK, P, N).transpose(1, 0, 2))
    return {
        "xTr": xTr,
        "mask": np.ascontiguousarray(mask[b].reshape(1, N).astype(np.int32)),
        "wq": Wq, "wk": Wk, "wv": Wv, "wo": Wo,
        "w1": W1, "w2": W2,
        "bq": bq, "bk": bk, "bv": bv,
        "b1": b1, "b2": b2, "bo": bo,
        "sel1": sel1, "sel2": sel2, "ones": ones_r,
    }


def kernel(x, mask, Wq, bq, Wk, bk, Wv, bv, W1, b1, W2, b2, Wo, bo,
           trace=False):
    bf = ml_dtypes.bfloat16
    x = np.asarray(x, dtype=np.float32)
    mask = np.asarray(mask)

    def wprep(w):  # [C, C] -> [P, CK, C] bf16 (lhsT/rhs chunk layout)
        w = np.asarray(w, dtype=np.float32).astype(bf)
        return np.ascontiguousarray(w.reshape(CK, P, C).transpose(1, 0, 2))

    def bprep(v):  # [C] -> [P, CK] f32
        v = np.asarray(v, dtype=np.float32)
        return np.ascontiguousarray(v.reshape(CK, P).T)

    Wq_p, Wk_p, Wv_p, Wo_p = wprep(Wq), wprep(Wk), wprep(Wv), wprep(Wo)
    W1_p = np.asarray(W1, np.float32).astype(bf)
    W2_p = np.asarray(W2, np.float32).astype(bf)
    bq_p, bk_p, bv_p = bprep(bq), bprep(bk), bprep(bv)
    b1_p = np.asarray(b1, np.float32).reshape(2 * D, 1)
    b2_p = np.asarray(b2, np.float32).reshape(H, 1)
    bo_p = np.asarray(bo, np.float32).astype(bf).reshape(1, C)

    # sel1[p, ci, h] = SCALE where channel (ci*128+p) belongs to head h
    cidx = np.arange(C)
    head_of = cidx // D
    sel1 = np.zeros((C, H), np.float32)
    sel1[cidx, head_of] = SCALE
    sel1 = np.ascontiguousarray(
        sel1.reshape(CK, P, H).transpose(1, 0, 2)).astype(bf)
    sel2 = np.zeros((H, C), np.float32)
    sel2[head_of, cidx] = 1.0
    sel2 = np.ascontiguousarray(sel2.reshape(H, CK, P)).astype(bf)
    ones_r = np.ones((1, P), np.float32).astype(bf)

    nc = _build()
    in_maps = [
        _prep_core_inputs(b, x, mask, Wq_p, bq_p, Wk_p, bk_p, Wv_p, bv_p,
                          W1_p, b1_p, W2_p, b2_p, Wo_p, bo_p,
                          sel1, sel2, ones_r)
        for b in range(B)
    ]
    res = run_bass_kernel_spmd(nc, in_maps, core_ids=list(range(B)),
                               trace=trace)
    out = np.stack([res.results[b]["out"] for b in range(B)], axis=0)
    if trace:
        kernel.last_exec_time_ns = res.exec_time_ns
        kernel.last_results = res
    return out


# revision 19
# speedup vs baseline: 1.0006x; 1.0006x over previous
"""HadamardAttention Trainium2 kernel — 8-core data-parallel over batch.

Per core (one batch element b), everything in "transposed" activation
layout [C on partitions, N on free dim]:

  phase A: qT/kT projections -> Hadamard product -> per-head reduction
           (selector matmul, SCALE folded in) -> tiny MLP -> masked
           scores awFull [H, N] (fp32)
  phase B: softmax over N (free dim) on [32, 4096]
  phase C: vT projection (x re-streamed), head-broadcast of weights
           (selector matmul), p2T = aw*vT, final out-projection which
           naturally restores natural [N, C] layout (p2T is the lhsT).

Host-side prep is layout-only (transpose/reshape) plus dtype casts to
bf16 for TensorE operands; all FLOPs happen on device.
"""
import sys

if "/opt/trn_rl_repo" not in sys.path:
    sys.path.insert(0, "/opt/trn_rl_repo")

import numpy as np
import ml_dtypes
from contextlib import ExitStack

import concourse.bass as bass
import concourse.bacc as bacc
import concourse.tile as tile
from concourse import mybir
from concourse.bass_utils import run_bass_kernel_spmd

# antenv.axon_hooks is absent in some images; shim it so trace=True can
# reach the NTFF profiler. Harmless no-op for trace=False runs.
try:
    from antenv.axon_hooks import get_axon_ntff_profile_hook  # noqa: F401
except ImportError:
    try:
        import types
        import antenv

        _hooks = types.ModuleType("antenv.axon_hooks")
        _hooks._hook = None
        _hooks.set_axon_ntff_profile_hook = lambda h: setattr(_hooks, "_hook", h)
        _hooks.get_axon_ntff_profile_hook = lambda: _hooks._hook
        sys.modules["antenv.axon_hooks"] = _hooks
        antenv.axon_hooks = _hooks
        from trn_agent_boot.trn_boot import _ntff_profile_via_ctypes

        _hooks.set_axon_ntff_profile_hook(
            _ntff_profile_via_ctypes("/opt/axon/libaxon_pjrt.so"))
    except Exception:
        pass

B, N, C, H, D = 8, 4096, 1024, 32, 32
SCALE = float(D) ** -0.5
P = 128
CK = C // P          # 8 chunks of the channel dim
NQ = 4               # token quarters
TQ = N // NQ         # 1024 tokens per quarter
TC = 512             # moving free dim per matmul
BF16 = mybir.dt.bfloat16
F32 = mybir.dt.float32
I32 = mybir.dt.int32
AF = mybir.ActivationFunctionType
ALU = mybir.AluOpType


def _build():
    nc = bacc.Bacc("TRN2", num_devices=8)

    xTr = nc.declare_dram_parameter("xTr", [P, CK, N], BF16, isOutput=False)
    mask = nc.declare_dram_parameter("mask", [1, N], I32, isOutput=False)
    wq = nc.declare_dram_parameter("wq", [P, CK, C], BF16, isOutput=False)
    wk = nc.declare_dram_parameter("wk", [P, CK, C], BF16, isOutput=False)
    wv = nc.declare_dram_parameter("wv", [P, CK, C], BF16, isOutput=False)
    wo = nc.declare_dram_parameter("wo", [P, CK, C], BF16, isOutput=False)
    w1 = nc.declare_dram_parameter("w1", [H, 2 * D], BF16, isOutput=False)
    w2 = nc.declare_dram_parameter("w2", [2 * D, H], BF16, isOutput=False)
    bq = nc.declare_dram_parameter("bq", [P, CK], F32, isOutput=False)
    bk = nc.declare_dram_parameter("bk", [P, CK], F32, isOutput=False)
    bv = nc.declare_dram_parameter("bv", [P, CK], F32, isOutput=False)
    b1 = nc.declare_dram_parameter("b1", [2 * D, 1], F32, isOutput=False)
    b2 = nc.declare_dram_parameter("b2", [H, 1], F32, isOutput=False)
    bo = nc.declare_dram_parameter("bo", [1, C], BF16, isOutput=False)
    sel1 = nc.declare_dram_parameter("sel1", [P, CK, H], BF16, isOutput=False)
    sel2 = nc.declare_dram_parameter("sel2", [H, CK, P], BF16, isOutput=False)
    ones = nc.declare_dram_parameter("ones", [1, P], BF16, isOutput=False)
    out = nc.declare_dram_parameter("out", [N, C], F32, isOutput=True)

    with tile.TileContext(nc) as tc:
        with ExitStack() as ctx:
            wpool = ctx.enter_context(tc.tile_pool(name="wpool", bufs=1))
            const = ctx.enter_context(tc.tile_pool(name="const", bufs=1))
            narrow = ctx.enter_context(tc.tile_pool(name="narrow", bufs=1))
            small = ctx.enter_context(tc.tile_pool(name="small", bufs=2))
            xin = ctx.enter_context(tc.tile_pool(name="xin", bufs=2))
            ppool = ctx.enter_context(tc.tile_pool(name="ppool", bufs=2))
            qv = ctx.enter_context(tc.tile_pool(name="qv", bufs=4))
            ypool = ctx.enter_context(tc.tile_pool(name="ypool", bufs=4))
            bank = ctx.enter_context(
                tc.tile_pool(name="bank", bufs=5, space="PSUM"))
            sbank = ctx.enter_context(
                tc.tile_pool(name="sbank", bufs=3, space="PSUM"))

            # ---- constants / weights -------------------------------------
            wq_sb = wpool.tile([P, CK, C], BF16, tag="wq")
            wk_sb = wpool.tile([P, CK, C], BF16, tag="wk")
            wv_sb = wpool.tile([P, CK, C], BF16, tag="wv")
            wo_sb = wpool.tile([P, CK, C], BF16, tag="wo")
            # startup-critical loads first, interleaved per-kc so the first
            # accumulation group's operands land ASAP: xt(q0) + wq, then wk.
            xt0 = xin.tile([P, CK, TQ], BF16, tag="xin", name="xt0")
            for kc in range(CK):
                nc.sync.dma_start(out=xt0[:, kc, :], in_=xTr[:, kc, 0:TQ])
                nc.sync.dma_start(out=wq_sb[:, kc, :], in_=wq[:, kc, :])
            for kc in range(CK):
                nc.sync.dma_start(out=wk_sb[:, kc, :], in_=wk[:, kc, :])
            w1_sb = const.tile([H, 2 * D], BF16, tag="w1")
            w2_sb = const.tile([2 * D, H], BF16, tag="w2")
            sel1_sb = const.tile([P, CK, H], BF16, tag="sel1")
            sel2_sb = const.tile([H, CK, P], BF16, tag="sel2")
            bq_sb = const.tile([P, CK], F32, tag="bq")
            bk_sb = const.tile([P, CK], F32, tag="bk")
            bv_sb = const.tile([P, CK], F32, tag="bv")
            b1_sb = const.tile([2 * D, 1], F32, tag="b1")
            b2_sb = const.tile([H, 1], F32, tag="b2")
            bo_sb = const.tile([1, C], BF16, tag="bo")
            ones_sb = const.tile([1, P], BF16, tag="ones")
            for t_sb, t_dr in ((w1_sb, w1), (w2_sb, w2), (sel1_sb, sel1),
                               (sel2_sb, sel2), (bq_sb, bq), (bk_sb, bk),
                               (bv_sb, bv), (b1_sb, b1), (b2_sb, b2),
                               (bo_sb, bo), (ones_sb, ones)):
                nc.sync.dma_start(out=t_sb[:], in_=t_dr[:])

            # additive mask row: 0 where mask==1, -1e9 where mask==0.
            # mask_sb borrows an xin slot (same byte size as the bf16 tiles).
            mask_sb = xin.tile([P, CK, TQ // 2], I32, tag="xin")
            mask_flat = mask_sb[:1, :, :].rearrange("p a b -> p (a b)")
            nc.sync.dma_start(out=mask_flat[:, :N], in_=mask[:, :])
            madd = narrow.tile([1, N], BF16, tag="madd")
            nc.vector.tensor_scalar(
                out=madd[:], in0=mask_flat[:, :N],
                scalar1=1e9, scalar2=-1e9, op0=ALU.mult, op1=ALU.add)

            # bo replicated across partitions via a step-0 DMA broadcast,
            # so the out-projection bias is a DVE add (not PE matmuls).
            bo_rep = const.tile([P, C], BF16, tag="bo_rep")
            bo_ap = bo[:, :]
            bo_bcast = bass.AP(tensor=bo_ap.tensor, offset=bo_ap.offset,
                               ap=[[0, P], list(bo_ap.ap)[1]])
            nc.gpsimd.dma_start(out=bo_rep[:], in_=bo_bcast)

            awFull = narrow.tile([H, N], F32, tag="awFull")
            awn = narrow.tile([H, N], BF16, tag="awn")
            maxP = narrow.tile([H, 2 * NQ], F32, tag="maxP")
            sumP = narrow.tile([H, 2 * NQ], F32, tag="sumP")
            negmax = narrow.tile([H, 1], F32, tag="negmax")
            sums = narrow.tile([H, 1], F32, tag="sums")
            inv = narrow.tile([H, 1], F32, tag="inv")

            # ---- phase A: scores ----------------------------------------
            for iq in range(NQ):
                if iq == 0:
                    xt = xt0
                else:
                    xt = xin.tile([P, CK, TQ], BF16, tag="xin")
                    for kc in range(CK):
                        nc.sync.dma_start(
                            out=xt[:, kc, :],
                            in_=xTr[:, kc, iq * TQ:(iq + 1) * TQ])
                pT = ppool.tile([P, CK, TQ], BF16, tag="pT")
                for mc in range(CK):
                    ps_q = [bank.tile([P, TC], F32, tag="bank",
                                      name=f"psq_{iq}_{mc}_{t}")
                            for t in range(2)]
                    for kc in range(CK):
                        for t in range(2):
                            nc.tensor.matmul(
                                ps_q[t][:],
                                wq_sb[:, kc, mc * P:(mc + 1) * P],
                                xt[:, kc, t * TC:(t + 1) * TC],
                                start=(kc == 0), stop=(kc == CK - 1))
                    q_mc = qv.tile([P, 2, TC], BF16, tag="qv")
                    for t in range(2):
                        nc.scalar.activation(
                            q_mc[:, t, :], ps_q[t][:], AF.Identity,
                            bias=bq_sb[:, mc:mc + 1])
                    ps_k = [bank.tile([P, TC], F32, tag="bank",
                                      name=f"psk_{iq}_{mc}_{t}")
                            for t in range(2)]
                    for kc in range(CK):
                        for t in range(2):
                            nc.tensor.matmul(
                                ps_k[t][:],
                                wk_sb[:, kc, mc * P:(mc + 1) * P],
                                xt[:, kc, t * TC:(t + 1) * TC],
                                start=(kc == 0), stop=(kc == CK - 1))
                    for t in range(2):
                        nc.vector.scalar_tensor_tensor(
                            out=pT[:, mc, t * TC:(t + 1) * TC],
                            in0=ps_k[t][:], scalar=bk_sb[:, mc:mc + 1],
                            in1=q_mc[:, t, :], op0=ALU.add, op1=ALU.mult)
                for t in range(2):
                    j = iq * 2 + t
                    ps_aw = sbank.tile([2 * D, TC], F32, tag="sbank")
                    for ci in range(CK):
                        nc.tensor.matmul(
                            ps_aw[:H, :], sel1_sb[:, ci, :],
                            pT[:, ci, t * TC:(t + 1) * TC],
                            start=(ci == 0), stop=(ci == CK - 1))
                    aw0 = small.tile([H, TC], BF16, tag="aw0")
                    nc.scalar.activation(aw0[:], ps_aw[:H, :], AF.Copy)
                    ps_a1 = sbank.tile([2 * D, TC], F32, tag="sbank")
                    nc.tensor.matmul(ps_a1[:], w1_sb[:], aw0[:],
                                     start=True, stop=True)
                    a1 = small.tile([2 * D, TC], BF16, tag="a1")
                    nc.scalar.activation(a1[:], ps_a1[:], AF.Relu,
                                         bias=b1_sb[:])
                    ps_aw2 = sbank.tile([2 * D, TC], F32, tag="sbank")
                    nc.tensor.matmul(ps_aw2[:H, :], w2_sb[:], a1[:],
                                     start=True, stop=False)
                    nc.tensor.matmul(ps_aw2[:H, :], ones_sb[:1, :H],
                                     madd[:1, j * TC:(j + 1) * TC],
                                     start=False, stop=True)
                    nc.scalar.activation(
                        awFull[:, j * TC:(j + 1) * TC], ps_aw2[:H, :],
                        AF.Identity, bias=b2_sb[:])
                    # flash-style: per-chunk -max, then exp in place
                    # with that max; global correction happens in phase B.
                    nc.vector.reduce_max(
                        out=maxP[:, j:j + 1],
                        in_=awFull[:, j * TC:(j + 1) * TC],
                        axis=mybir.AxisListType.X, negate=True)
                    nc.scalar.activation(awFull[:, j * TC:(j + 1) * TC],
                                         awFull[:, j * TC:(j + 1) * TC],
                                         AF.Exp, bias=maxP[:, j:j + 1],
                                         accum_out=sumP[:, j:j + 1])

            # ---- phase B: flash-softmax correction (tiny) ---------------
            # maxP holds -chunkmax; global negmax = min_j maxP[j].
            nc.vector.tensor_reduce(out=negmax[:], in_=maxP[:],
                                    axis=mybir.AxisListType.X,
                                    op=ALU.min)
            # corr[j] = exp(chunkmax_j - globalmax) = exp(-maxP_j + negmax)
            corr = narrow.tile([H, 2 * NQ], F32, tag="corr")
            nc.scalar.activation(corr[:], maxP[:], AF.Exp,
                                 bias=negmax[:], scale=-1.0)
            wsum = narrow.tile([H, 2 * NQ], F32, tag="wsum")
            nc.vector.tensor_mul(wsum[:], sumP[:], corr[:])
            nc.vector.reduce_sum(out=sums[:], in_=wsum[:],
                                 axis=mybir.AxisListType.X)
            nc.vector.reciprocal(out=inv[:], in_=sums[:])
            sfac = narrow.tile([H, 2 * NQ], F32, tag="sfac")
            nc.vector.tensor_scalar(out=sfac[:], in0=corr[:],
                                    scalar1=inv[:], scalar2=None,
                                    op0=ALU.mult)
            for j in range(2 * NQ):
                nc.scalar.activation(awn[:, j * TC:(j + 1) * TC],
                                     awFull[:, j * TC:(j + 1) * TC],
                                     AF.Copy, scale=sfac[:, j:j + 1])

            # ---- phase C: v, weighting, out-projection ------------------
            # wv/wo stream in during phase A compute
            for t_sb, t_dr in ((wv_sb, wv), (wo_sb, wo)):
                for kc in range(CK):
                    nc.sync.dma_start(out=t_sb[:, kc, :],
                                      in_=t_dr[:, kc, :])
            for iq in range(NQ):
                xt = xin.tile([P, CK, TQ], BF16, tag="xin")
                for kc in range(CK):
                    nc.sync.dma_start(
                        out=xt[:, kc, :],
                        in_=xTr[:, kc, iq * TQ:(iq + 1) * TQ])
                p2 = ppool.tile([P, CK, TQ], BF16, tag="pT")
                # software pipeline: v-matmuls for mc run 2 iterations ahead
                # of the awn-dependent awb/p2 stage, so the PE has ~32 MMs
                # of cover while the softmax chain finishes.
                psv = {}

                def v_stage(mc, iq=iq, xt=xt, psv=psv):
                    psv[mc] = [bank.tile([P, TC], F32, tag="bank",
                                         name=f"psv_{iq}_{mc}_{t}")
                               for t in range(2)]
                    for kc in range(CK):
                        for t in range(2):
                            nc.tensor.matmul(
                                psv[mc][t][:],
                                wv_sb[:, kc, mc * P:(mc + 1) * P],
                                xt[:, kc, t * TC:(t + 1) * TC],
                                start=(kc == 0), stop=(kc == CK - 1))

                def awb_stage(mc, iq=iq, p2=p2, psv=psv):
                    awb_mc = qv.tile([P, 2, TC], BF16, tag="qv",
                                     name=f"awb_{iq}_{mc}")
                    for t in range(2):
                        j = iq * 2 + t
                        ps_awb = sbank.tile([P, TC], F32, tag="sbank",
                                            name=f"psawb_{iq}_{mc}_{t}")
                        nc.tensor.matmul(
                            ps_awb[:], sel2_sb[:, mc, :],
                            awn[:, j * TC:(j + 1) * TC],
                            start=True, stop=True)
                        nc.vector.tensor_copy(out=awb_mc[:, t, :],
                                              in_=ps_awb[:])
                    for t in range(2):
                        nc.vector.scalar_tensor_tensor(
                            out=p2[:, mc, t * TC:(t + 1) * TC],
                            in0=psv[mc][t][:], scalar=bv_sb[:, mc:mc + 1],
                            in1=awb_mc[:, t, :], op0=ALU.add, op1=ALU.mult)
                    del psv[mc]

                LOOKAHEAD = 1
                for mc in range(CK):
                    v_stage(mc)
                    if mc >= LOOKAHEAD:
                        awb_stage(mc - LOOKAHEAD)
                for mc in range(CK - LOOKAHEAD, CK):
                    awb_stage(mc)
                for nt in range(TQ // P):
                    n0 = iq * TQ + nt * P
                    for co in range(2):
                        ps_y = bank.tile([P, TC], F32, tag="bank")
                        for ci in range(CK):
                            nc.tensor.matmul(
                                ps_y[:], p2[:, ci, nt * P:(nt + 1) * P],
                                wo_sb[:, ci, co * TC:(co + 1) * TC],
                                start=(ci == 0), stop=(ci == CK - 1))
                        y_sb = ypool.tile([P, TC], F32, tag="y")
                        nc.vector.tensor_add(
                            y_sb[:], ps_y[:],
                            bo_rep[:, co * TC:(co + 1) * TC])
                        nc.sync.dma_start(
                            out=out[n0:n0 + P, co * TC:(co + 1) * TC],
                            in_=y_sb[:])
    nc.finalize()
    return nc


def _prep_core_inputs(b, x, mask, Wq, bq, Wk, bk, Wv, bv, W1, b1, W2, b2,
                      Wo, bo, sel1, sel2, ones_r):
    bf = ml_dtypes.bfloat16
    xT = np.ascontiguousarray(x[b].T).astype(bf)            # [C, N]
    xTr = np.ascontiguousarray(xT.reshape(CK, P, N).transpose(1, 0, 2))
    return {
        "xTr": xTr,
        "mask": np.ascontiguousarray(mask[b].reshape(1, N).astype(np.int32)),
        "wq": Wq, "wk": Wk, "wv": Wv, "wo": Wo,
        "w1": W1, "w2": W2,
        "bq": bq, "bk": bk, "bv": bv,
        "b1": b1, "b2": b2, "bo": bo,
        "sel1": sel1, "sel2": sel2, "ones": ones_r,
    }


def kernel(x, mask, Wq, bq, Wk, bk, Wv, bv, W1, b1, W2, b2, Wo, bo,
           trace=False):
    bf = ml_dtypes.bfloat16
    x = np.asarray(x, dtype=np.float32)
    mask = np.asarray(mask)

    def wprep(w):  # [C, C] -> [P, CK, C] bf16 (lhsT/rhs chunk layout)
        w = np.asarray(w, dtype=np.float32).astype(bf)
        return np.ascontiguousarray(w.reshape(CK, P, C).transpose(1, 0, 2))

    def bprep(v):  # [C] -> [P, CK] f32
        v = np.asarray(v, dtype=np.float32)
        return np.ascontiguousarray(v.reshape(CK, P).T)

    Wq_p, Wk_p, Wv_p, Wo_p = wprep(Wq), wprep(Wk), wprep(Wv), wprep(Wo)
    W1_p = np.asarray(W1, np.float32).astype(bf)
    W2_p = np.asarray(W2, np.float32).astype(bf)
    bq_p, bk_p, bv_p = bprep(bq), bprep(bk), bprep(bv)
    b1_p = np.asarray(b1, np.float32).reshape(2 * D, 1)
    b2_p = np.asarray(b2, np.float32).reshape(H, 1)
    bo_p = np.asarray(bo, np.float32).astype(bf).reshape(1, C)

    # sel1[p, ci, h] = SCALE where channel (ci*128+p) belongs to head h
    cidx = np.arange(C)
    head_of = cidx // D
    sel1 = np.zeros((C, H), np.float32)
    sel1[cidx, head_of] = SCALE
    sel1 = np.ascontiguousarray(
        sel1.reshape(CK, P, H).transpose(1, 0, 2)).astype(bf)
    sel2 = np.zeros((H, C), np.float32)
    sel2[head_of, cidx] = 1.0
    sel2 = np.ascontiguousarray(sel2.reshape(H, CK, P)).astype(bf)
    ones_r = np.ones((1, P), np.float32).astype(bf)

    nc = _build()
    in_maps = [
        _prep_core_inputs(b, x, mask, Wq_p, bq_p, Wk_p, bk_p, Wv_p, bv_p,
                          W1_p, b1_p, W2_p, b2_p, Wo_p, bo_p,
                          sel1, sel2, ones_r)
        for b in range(B)
    ]
    res = run_bass_kernel_spmd(nc, in_maps, core_ids=list(range(B)),
                               trace=trace)
    out = np.stack([res.results[b]["out"] for b in range(B)], axis=0)
    if trace:
        kernel.last_exec_time_ns = res.exec_time_ns
        kernel.last_results = res
    return out


# revision 21
# speedup vs baseline: 1.0038x; 1.0032x over previous
"""HadamardAttention Trainium2 kernel — 8-core data-parallel over batch.

Per core (one batch element b), everything in "transposed" activation
layout [C on partitions, N on free dim]:

  phase A: qT/kT projections -> Hadamard product -> per-head reduction
           (selector matmul, SCALE folded in) -> tiny MLP -> masked
           scores awFull [H, N] (fp32)
  phase B: softmax over N (free dim) on [32, 4096]
  phase C: vT projection (x re-streamed), head-broadcast of weights
           (selector matmul), p2T = aw*vT, final out-projection which
           naturally restores natural [N, C] layout (p2T is the lhsT).

Host-side prep is layout-only (transpose/reshape) plus dtype casts to
bf16 for TensorE operands; all FLOPs happen on device.
"""
import sys

if "/opt/trn_rl_repo" not in sys.path:
    sys.path.insert(0, "/opt/trn_rl_repo")

import numpy as np
import ml_dtypes
from contextlib import ExitStack

import concourse.bass as bass
import concourse.bacc as bacc
import concourse.tile as tile
from concourse import mybir
from concourse.bass_utils import run_bass_kernel_spmd

# antenv.axon_hooks is absent in some images; shim it so trace=True can
# reach the NTFF profiler. Harmless no-op for trace=False runs.
try:
    from antenv.axon_hooks import get_axon_ntff_profile_hook  # noqa: F401
except ImportError:
    try:
        import types
        import antenv

        _hooks = types.ModuleType("antenv.axon_hooks")
        _hooks._hook = None
        _hooks.set_axon_ntff_profile_hook = lambda h: setattr(_hooks, "_hook", h)
        _hooks.get_axon_ntff_profile_hook = lambda: _hooks._hook
        sys.modules["antenv.axon_hooks"] = _hooks
        antenv.axon_hooks = _hooks
        from trn_agent_boot.trn_boot import _ntff_profile_via_ctypes

        _hooks.set_axon_ntff_profile_hook(
            _ntff_profile_via_ctypes("/opt/axon/libaxon_pjrt.so"))
    except Exception:
        pass

B, N, C, H, D = 8, 4096, 1024, 32, 32
SCALE = float(D) ** -0.5
P = 128
CK = C // P          # 8 chunks of the channel dim
NQ = 4               # token quarters
TQ = N // NQ         # 1024 tokens per quarter
TC = 512             # moving free dim per matmul
BF16 = mybir.dt.bfloat16
F32 = mybir.dt.float32
I32 = mybir.dt.int32
AF = mybir.ActivationFunctionType
ALU = mybir.AluOpType


def _build():
    nc = bacc.Bacc("TRN2", num_devices=8)

    xTr = nc.declare_dram_parameter("xTr", [P, CK, N], BF16, isOutput=False)
    mask = nc.declare_dram_parameter("mask", [1, N], I32, isOutput=False)
    wq = nc.declare_dram_parameter("wq", [P, CK, C], BF16, isOutput=False)
    wk = nc.declare_dram_parameter("wk", [P, CK, C], BF16, isOutput=False)
    wv = nc.declare_dram_parameter("wv", [P, CK, C], BF16, isOutput=False)
    wo = nc.declare_dram_parameter("wo", [P, CK, C], BF16, isOutput=False)
    w1 = nc.declare_dram_parameter("w1", [H, 2 * D], BF16, isOutput=False)
    w2 = nc.declare_dram_parameter("w2", [2 * D, H], BF16, isOutput=False)
    bq = nc.declare_dram_parameter("bq", [P, CK], F32, isOutput=False)
    bk = nc.declare_dram_parameter("bk", [P, CK], F32, isOutput=False)
    bv = nc.declare_dram_parameter("bv", [P, CK], F32, isOutput=False)
    b1 = nc.declare_dram_parameter("b1", [2 * D, 1], F32, isOutput=False)
    b2 = nc.declare_dram_parameter("b2", [H, 1], F32, isOutput=False)
    bo = nc.declare_dram_parameter("bo", [1, C], BF16, isOutput=False)
    sel1 = nc.declare_dram_parameter("sel1", [P, CK, H], BF16, isOutput=False)
    sel2 = nc.declare_dram_parameter("sel2", [H, CK, P], BF16, isOutput=False)
    ones = nc.declare_dram_parameter("ones", [1, P], BF16, isOutput=False)
    out = nc.declare_dram_parameter("out", [N, C], F32, isOutput=True)

    with tile.TileContext(nc) as tc:
        with ExitStack() as ctx:
            wpool = ctx.enter_context(tc.tile_pool(name="wpool", bufs=1))
            const = ctx.enter_context(tc.tile_pool(name="const", bufs=1))
            narrow = ctx.enter_context(tc.tile_pool(name="narrow", bufs=1))
            small = ctx.enter_context(tc.tile_pool(name="small", bufs=2))
            xin = ctx.enter_context(tc.tile_pool(name="xin", bufs=2))
            ppool = ctx.enter_context(tc.tile_pool(name="ppool", bufs=2))
            qv = ctx.enter_context(tc.tile_pool(name="qv", bufs=4))
            ypool = ctx.enter_context(tc.tile_pool(name="ypool", bufs=4))
            bank = ctx.enter_context(
                tc.tile_pool(name="bank", bufs=5, space="PSUM"))
            sbank = ctx.enter_context(
                tc.tile_pool(name="sbank", bufs=3, space="PSUM"))

            # ---- constants / weights -------------------------------------
            wq_sb = wpool.tile([P, CK, C], BF16, tag="wq")
            wk_sb = wpool.tile([P, CK, C], BF16, tag="wk")
            wv_sb = wpool.tile([P, CK, C], BF16, tag="wv")
            wo_sb = wpool.tile([P, CK, C], BF16, tag="wo")
            # startup-critical loads first, interleaved per-kc so the first
            # accumulation group's operands land ASAP: xt(q0) + wq, then wk.
            xt0 = xin.tile([P, CK, TQ], BF16, tag="xin", name="xt0")
            for kc in range(CK):
                nc.sync.dma_start(out=xt0[:, kc, :], in_=xTr[:, kc, 0:TQ])
                nc.sync.dma_start(out=wq_sb[:, kc, :], in_=wq[:, kc, :])
            for kc in range(CK):
                nc.sync.dma_start(out=wk_sb[:, kc, :], in_=wk[:, kc, :])
            w1_sb = const.tile([H, 2 * D], BF16, tag="w1")
            w2_sb = const.tile([2 * D, H], BF16, tag="w2")
            sel1_sb = const.tile([P, CK, H], BF16, tag="sel1")
            sel2_sb = const.tile([H, CK, P], BF16, tag="sel2")
            bq_sb = const.tile([P, CK], F32, tag="bq")
            bk_sb = const.tile([P, CK], F32, tag="bk")
            bv_sb = const.tile([P, CK], F32, tag="bv")
            b1_sb = const.tile([2 * D, 1], F32, tag="b1")
            b2_sb = const.tile([H, 1], F32, tag="b2")
            bo_sb = const.tile([1, C], BF16, tag="bo")
            ones_sb = const.tile([1, P], BF16, tag="ones")
            for t_sb, t_dr in ((w1_sb, w1), (w2_sb, w2), (sel1_sb, sel1),
                               (sel2_sb, sel2), (bq_sb, bq), (bk_sb, bk),
                               (bv_sb, bv), (b1_sb, b1), (b2_sb, b2),
                               (bo_sb, bo), (ones_sb, ones)):
                nc.sync.dma_start(out=t_sb[:], in_=t_dr[:])

            # additive mask row: 0 where mask==1, -1e9 where mask==0.
            # mask_sb borrows an xin slot (same byte size as the bf16 tiles).
            mask_sb = xin.tile([P, CK, TQ // 2], I32, tag="xin")
            mask_flat = mask_sb[:1, :, :].rearrange("p a b -> p (a b)")
            nc.sync.dma_start(out=mask_flat[:, :N], in_=mask[:, :])
            madd = narrow.tile([1, N], BF16, tag="madd")
            nc.vector.tensor_scalar(
                out=madd[:], in0=mask_flat[:, :N],
                scalar1=1e9, scalar2=-1e9, op0=ALU.mult, op1=ALU.add)

            # bo replicated across partitions via a step-0 DMA broadcast,
            # so the out-projection bias is a DVE add (not PE matmuls).
            bo_rep = const.tile([P, C], BF16, tag="bo_rep")
            bo_ap = bo[:, :]
            bo_bcast = bass.AP(tensor=bo_ap.tensor, offset=bo_ap.offset,
                               ap=[[0, P], list(bo_ap.ap)[1]])
            nc.gpsimd.dma_start(out=bo_rep[:], in_=bo_bcast)

            awFull = narrow.tile([H, N], F32, tag="awFull")
            awn = narrow.tile([H, N], BF16, tag="awn")
            maxP = narrow.tile([H, 2 * NQ], F32, tag="maxP")
            sumP = narrow.tile([H, 2 * NQ], F32, tag="sumP")
            negmax = narrow.tile([H, 1], F32, tag="negmax")
            sums = narrow.tile([H, 1], F32, tag="sums")
            inv = narrow.tile([H, 1], F32, tag="inv")

            # ---- phase A: scores ----------------------------------------
            for iq in range(NQ):
                if iq == 0:
                    xt = xt0
                else:
                    xt = xin.tile([P, CK, TQ], BF16, tag="xin")
                    for kc in range(CK):
                        nc.sync.dma_start(
                            out=xt[:, kc, :],
                            in_=xTr[:, kc, iq * TQ:(iq + 1) * TQ])
                pT = ppool.tile([P, CK, TQ], BF16, tag="pT")
                for mc in range(CK):
                    ps_q = [bank.tile([P, TC], F32, tag="bank",
                                      name=f"psq_{iq}_{mc}_{t}")
                            for t in range(2)]
                    for kc in range(CK):
                        for t in range(2):
                            nc.tensor.matmul(
                                ps_q[t][:],
                                wq_sb[:, kc, mc * P:(mc + 1) * P],
                                xt[:, kc, t * TC:(t + 1) * TC],
                                start=(kc == 0), stop=(kc == CK - 1))
                    q_mc = qv.tile([P, 2, TC], BF16, tag="qv")
                    for t in range(2):
                        nc.scalar.activation(
                            q_mc[:, t, :], ps_q[t][:], AF.Identity,
                            bias=bq_sb[:, mc:mc + 1])
                    ps_k = [bank.tile([P, TC], F32, tag="bank",
                                      name=f"psk_{iq}_{mc}_{t}")
                            for t in range(2)]
                    for kc in range(CK):
                        for t in range(2):
                            nc.tensor.matmul(
                                ps_k[t][:],
                                wk_sb[:, kc, mc * P:(mc + 1) * P],
                                xt[:, kc, t * TC:(t + 1) * TC],
                                start=(kc == 0), stop=(kc == CK - 1))
                    for t in range(2):
                        nc.vector.scalar_tensor_tensor(
                            out=pT[:, mc, t * TC:(t + 1) * TC],
                            in0=ps_k[t][:], scalar=bk_sb[:, mc:mc + 1],
                            in1=q_mc[:, t, :], op0=ALU.add, op1=ALU.mult)
                for t in range(2):
                    j = iq * 2 + t
                    ps_aw = sbank.tile([2 * D, TC], F32, tag="sbank")
                    for ci in range(CK):
                        nc.tensor.matmul(
                            ps_aw[:H, :], sel1_sb[:, ci, :],
                            pT[:, ci, t * TC:(t + 1) * TC],
                            start=(ci == 0), stop=(ci == CK - 1))
                    aw0 = small.tile([H, TC], BF16, tag="aw0")
                    nc.scalar.activation(aw0[:], ps_aw[:H, :], AF.Copy)
                    ps_a1 = sbank.tile([2 * D, TC], F32, tag="sbank")
                    nc.tensor.matmul(ps_a1[:], w1_sb[:], aw0[:],
                                     start=True, stop=True)
                    a1 = small.tile([2 * D, TC], BF16, tag="a1")
                    nc.scalar.activation(a1[:], ps_a1[:], AF.Relu,
                                         bias=b1_sb[:])
                    ps_aw2 = sbank.tile([2 * D, TC], F32, tag="sbank")
                    nc.tensor.matmul(ps_aw2[:H, :], w2_sb[:], a1[:],
                                     start=True, stop=False)
                    nc.tensor.matmul(ps_aw2[:H, :], ones_sb[:1, :H],
                                     madd[:1, j * TC:(j + 1) * TC],
                                     start=False, stop=True)
                    nc.scalar.activation(
                        awFull[:, j * TC:(j + 1) * TC], ps_aw2[:H, :],
                        AF.Identity, bias=b2_sb[:])
                    # flash-style: per-chunk -max, then exp in place
                    # with that max; global correction happens in phase B.
                    nc.vector.reduce_max(
                        out=maxP[:, j:j + 1],
                        in_=awFull[:, j * TC:(j + 1) * TC],
                        axis=mybir.AxisListType.X, negate=True)
                    nc.scalar.activation(awFull[:, j * TC:(j + 1) * TC],
                                         awFull[:, j * TC:(j + 1) * TC],
                                         AF.Exp, bias=maxP[:, j:j + 1],
                                         accum_out=sumP[:, j:j + 1])

            # ---- phase B: flash-softmax correction (tiny) ---------------
            # maxP holds -chunkmax; global negmax = min_j maxP[j].
            nc.vector.tensor_reduce(out=negmax[:], in_=maxP[:],
                                    axis=mybir.AxisListType.X,
                                    op=ALU.min)
            # corr[j] = exp(chunkmax_j - globalmax) = exp(-maxP_j + negmax)
            corr = narrow.tile([H, 2 * NQ], F32, tag="corr")
            nc.scalar.activation(corr[:], maxP[:], AF.Exp,
                                 bias=negmax[:], scale=-1.0)
            wsum = narrow.tile([H, 2 * NQ], F32, tag="wsum")
            nc.vector.tensor_mul(wsum[:], sumP[:], corr[:])
            nc.vector.reduce_sum(out=sums[:], in_=wsum[:],
                                 axis=mybir.AxisListType.X)
            nc.vector.reciprocal(out=inv[:], in_=sums[:])
            sfac = narrow.tile([H, 2 * NQ], F32, tag="sfac")
            nc.vector.tensor_scalar(out=sfac[:], in0=corr[:],
                                    scalar1=inv[:], scalar2=None,
                                    op0=ALU.mult)
            for j in range(2 * NQ):
                nc.scalar.activation(awn[:, j * TC:(j + 1) * TC],
                                     awFull[:, j * TC:(j + 1) * TC],
                                     AF.Copy, scale=sfac[:, j:j + 1])

            # ---- phase C: v, weighting, out-projection ------------------
            # wv/wo stream in during phase A compute
            for t_sb, t_dr in ((wv_sb, wv), (wo_sb, wo)):
                for kc in range(CK):
                    nc.sync.dma_start(out=t_sb[:, kc, :],
                                      in_=t_dr[:, kc, :])
            for iq in range(NQ):
                xt = xin.tile([P, CK, TQ], BF16, tag="xin")
                for kc in range(CK):
                    nc.sync.dma_start(
                        out=xt[:, kc, :],
                        in_=xTr[:, kc, iq * TQ:(iq + 1) * TQ])
                p2 = ppool.tile([P, CK, TQ], BF16, tag="pT")
                # software pipeline: v-matmuls for mc run 2 iterations ahead
                # of the awn-dependent awb/p2 stage, so the PE has ~32 MMs
                # of cover while the softmax chain finishes.
                psv = {}

                def v_stage(mc, iq=iq, xt=xt, psv=psv):
                    psv[mc] = [bank.tile([P, TC], F32, tag="bank",
                                         name=f"psv_{iq}_{mc}_{t}")
                               for t in range(2)]
                    for kc in range(CK):
                        for t in range(2):
                            nc.tensor.matmul(
                                psv[mc][t][:],
                                wv_sb[:, kc, mc * P:(mc + 1) * P],
                                xt[:, kc, t * TC:(t + 1) * TC],
                                start=(kc == 0), stop=(kc == CK - 1))

                def awb_stage(mc, iq=iq, p2=p2, psv=psv):
                    awb_mc = qv.tile([P, 2, TC], BF16, tag="qv",
                                     name=f"awb_{iq}_{mc}")
                    for t in range(2):
                        j = iq * 2 + t
                        ps_awb = sbank.tile([P, TC], F32, tag="sbank",
                                            name=f"psawb_{iq}_{mc}_{t}")
                        nc.tensor.matmul(
                            ps_awb[:], sel2_sb[:, mc, :],
                            awn[:, j * TC:(j + 1) * TC],
                            start=True, stop=True)
                        nc.vector.tensor_copy(out=awb_mc[:, t, :],
                                              in_=ps_awb[:])
                    for t in range(2):
                        nc.vector.scalar_tensor_tensor(
                            out=p2[:, mc, t * TC:(t + 1) * TC],
                            in0=psv[mc][t][:], scalar=bv_sb[:, mc:mc + 1],
                            in1=awb_mc[:, t, :], op0=ALU.add, op1=ALU.mult)
                    del psv[mc]

                LOOKAHEAD = 1
                for mc in range(CK):
                    v_stage(mc)
                    if mc >= LOOKAHEAD:
                        awb_stage(mc - LOOKAHEAD)
                for mc in range(CK - LOOKAHEAD, CK):
                    awb_stage(mc)
                for nt in range(TQ // P):
                    n0 = iq * TQ + nt * P
                    for co in range(2):
                        ps_y = bank.tile([P, TC], F32, tag="bank")
                        for ci in range(CK):
                            nc.tensor.matmul(
                                ps_y[:], p2[:, ci, nt * P:(nt + 1) * P],
                                wo_sb[:, ci, co * TC:(co + 1) * TC],
                                start=(ci == 0), stop=(ci == CK - 1))
                        y_sb = ypool.tile([P, TC], F32, tag="y")
                        nc.vector.tensor_add(
                            y_sb[:], ps_y[:],
                            bo_rep[:, co * TC:(co + 1) * TC])
                        nc.sync.dma_start(
                            out=out[n0:n0 + P, co * TC:(co + 1) * TC],
                            in_=y_sb[:])
    nc.finalize()
    return nc


def _prep_core_inputs(b, x, mask, Wq, bq, Wk, bk, Wv, bv, W1, b1, W2, b2,
                      Wo, bo, sel1, sel2, ones_r):
    bf = ml_dtypes.bfloat16
    xT = np.ascontiguousarray(x[b].T).astype(bf)            # [C, N]
    xTr = np.ascontiguousarray(xT.reshape(CK, P, N).transpose(1, 0, 2))
    return {
        "xTr": xTr,
        "mask": np.ascontiguousarray(mask[b].reshape(1, N).astype(np.int32)),
        "wq": Wq, "wk": Wk, "wv": Wv, "wo": Wo,
        "w1": W1, "w2": W2,
        "bq": bq, "bk": bk, "bv": bv,
        "b1": b1, "b2": b2, "bo": bo,
        "sel1": sel1, "sel2": sel2, "ones": ones_r,
    }


def kernel(x, mask, Wq, bq, Wk, bk, Wv, bv, W1, b1, W2, b2, Wo, bo,
           trace=False):
    bf = ml_dtypes.bfloat16
    x = np.asarray(x, dtype=np.float32)
    mask = np.asarray(mask)

    def wprep(w):  # [C, C] -> [P, CK, C] bf16 (lhsT/rhs chunk layout)
        w = np.asarray(w, dtype=np.float32).astype(bf)
        return np.ascontiguousarray(w.reshape(CK, P, C).transpose(1, 0, 2))

    def bprep(v):  # [C] -> [P, CK] f32
        v = np.asarray(v, dtype=np.float32)
        return np.ascontiguousarray(v.reshape(CK, P).T)

    Wq_p, Wk_p, Wv_p, Wo_p = wprep(Wq), wprep(Wk), wprep(Wv), wprep(Wo)
    W1_p = np.asarray(W1, np.float32).astype(bf)
    W2_p = np.asarray(W2, np.float32).astype(bf)
    bq_p, bk_p, bv_p = bprep(bq), bprep(bk), bprep(bv)
    b1_p = np.asarray(b1, np.float32).reshape(2 * D, 1)
    b2_p = np.asarray(b2, np.float32).reshape(H, 1)
    bo_p = np.asarray(bo, np.float32).astype(bf).reshape(1, C)

    # sel1[p, ci, h] = SCALE where channel (ci*128+p) belongs to head h
    cidx = np.arange(C)
    head_of = cidx // D
    sel1 = np.zeros((C, H), np.float32)
    sel1[cidx, head_of] = SCALE
    sel1 = np.ascontiguousarray(
        sel1.reshape(CK, P, H).transpose(1, 0, 2)).astype(bf)
    sel2 = np.zeros((H, C), np.float32)
    sel2[head_of, cidx] = 1.0
    sel2 = np.ascontiguousarray(sel2.reshape(H, CK, P)).astype(bf)
    ones_r = np.ones((1, P), np.float32).astype(bf)

    nc = _build()
    in_maps = [
        _prep_core_inputs(b, x, mask, Wq_p, bq_p, Wk_p, bk_p, Wv_p, bv_p,
                          W1_p, b1_p, W2_p, b2_p, Wo_p, bo_p,
                          sel1, sel2, ones_r)
        for b in range(B)
    ]
    res = run_bass_kernel_spmd(nc, in_maps, core_ids=list(range(B)),
                               trace=trace)
    out = np.stack([res.results[b]["out"] for b in range(B)], axis=0)
    if trace:
        kernel.last_exec_time_ns = res.exec_time_ns
        kernel.last_results = res
    return out
